# revision 1
# baseline (speedup 1.0000x reference)
"""Trainium2 Bass kernel for nn_DIT_11458972746143 (retrieval_knn).

B=16 batches sharded over 8 NeuronCores (2 per core). Per batch:
  1. KNN: PE computes t' = 2*x_i.x_j - |x_i|^2 - |x_j|^2 - 1e-7 + 0.1 per
     128-row block (K=5 augmented fp32 matmul). Mask (exclude d2<0.1) is one
     scalar_tensor_tensor: masked = min(t', -1e30*t'). Exact top-10 per row
     via DVE max8 / max_index / match_replace / max8 / max_index (matches
     jax.lax.top_k tie semantics).
  2. Index lists: max_index u32 outputs (k-major slots) -> f32 -> PE
     transpose -> u16 wrapped lists for gpsimd indirect_copy.
  3. Gather neighbor coords (src+tgt xyz) in stream layout, list position
     i = p*16 + T for point n = T*128 + p.
  4. PE-transpose gathered data to point-major layout.
  5. Triangle phase: 45 pairs/point, edge lengths from coordinates (matches
     reference rounding), sort3 min/max network, loss ratio, 10-smallest-of-45
     via max8 rounds, ACT sqrt + 2 Newton steps, mean, per-batch min,
     threshold loss-min < log(7/3)/30 (== 2*sigmoid(-30*(loss-min)) > 0.6).
"""

from contextlib import ExitStack

import numpy as np

import concourse.bass as bass
import concourse.tile as tile
from concourse import bacc, masks, mybir
from concourse.bass_utils import run_bass_kernel_spmd

F32 = mybir.dt.float32
U32 = mybir.dt.uint32
U16 = mybir.dt.uint16
OP = mybir.AluOpType
AX = mybir.AxisListType

N = 2048
NB = 16            # row blocks of 128
BPC = 2            # batches per core
K = 10
NPAIR = 45
BIGNEG = -1e30
C0 = float(np.float32(np.float64(0.1) - np.float64(1e-7)))
CTH = float(np.float32(np.log(np.float64(7.0) / 3.0) / 30.0))
EPS = 1e-6

_CACHE = {}


def rap(t, p_start, p_step, p_count, free_off, free_dims):
    """Raw AP over tile t: partitions [p_start::p_step] x free pattern."""
    base = t[:]
    pitch = base.ap[0][0]
    return bass.AP(
        tensor=t.tensor,
        offset=base.offset + p_start * pitch + free_off,
        ap=[[p_step * pitch, p_count]] + list(free_dims),
    )


def _build_setup(ctx, tc, pools, b, src_d, tgt_d, ident):
    nc = tc.nc
    sb = pools["sb"]

    # ---------------- Phase A: per-batch setup ----------------
    X = sb.tile([3, N], F32, tag="DAB")
    nc.sync.dma_start(X[:], src_d[b])
    ST = sb.tile([3, N], F32, tag="ETRI")
    nc.scalar.square(ST[:], X[:])
    T1 = sb.tile([1, N], F32, tag="SRT")
    T2 = sb.tile([1, N], F32, tag="DN")
    nc.sync.dma_start(T1[:], ST[1:2, :])
    nc.sync.dma_start(T2[:], ST[2:3, :])
    SQR = sb.tile([1, N], F32, tag="SM")
    nc.vector.tensor_tensor(SQR[:], ST[0:1, :], T1[:], OP.add)
    nc.vector.tensor_tensor(SQR[:], SQR[:], T2[:], OP.add)
    sq = SQR[:]

    ONES = sb.tile([1, N], F32, tag="G2")
    nc.vector.memset(ONES[:], 1.0)  # per-batch; dies before G2 write
    L4 = sb.tile([1, N], F32, tag="TL1")
    nc.vector.tensor_scalar(L4[:], sq, -1.0, C0, OP.mult, OP.add)
    NSQ = sb.tile([1, N], F32, tag="G1")
    nc.vector.tensor_scalar(NSQ[:], sq, -1.0, None, OP.mult)

    LT = sb.tile([5, N], F32, tag=f"LT{b}")
    nc.scalar.copy(LT[0:3, :], X[:])
    nc.sync.dma_start(LT[3:4, :], ONES[:])
    nc.sync.dma_start(LT[4:5, :], L4[:])
    RHS = sb.tile([5, N], F32, tag=f"RHS{b}")
    nc.scalar.mul(RHS[0:3, :], X[:], 2.0)
    nc.sync.dma_start(RHS[3:4, :], NSQ[:])
    nc.sync.dma_start(RHS[4:5, :], ONES[:])

    # gather table: rows 16g+c, c in 0..5 = (sx, sy, sz, tx, ty, tz)
    D6 = sb.tile([128, N], F32, tag=f"D6{b}")
    nc.gpsimd.memset(D6[:], 0.0)
    for g in range(8):
        nc.sync.dma_start(D6[16 * g : 16 * g + 3, :], src_d[b])
        nc.sync.dma_start(D6[16 * g + 3 : 16 * g + 6, :], tgt_d[b])
    return {"LT": LT, "RHS": RHS, "D6": D6}


def _build_main(ctx, tc, pools, b, st, out_d, ident):
    nc = tc.nc
    sb, sbk = pools["sb"], pools["sbk"]
    ps1, ps2 = pools["ps1"], pools["ps2"]
    LT, RHS, D6 = st["LT"], st["RHS"], st["D6"]

    # ---------------- Phase B: KNN per block ----------------
    IU32 = sb.tile([128, 256], U32, tag="IU32")  # slots k*16 + T
    for T in range(NB):
        pt = ps1.tile([128, N], F32, tag="knnpsum")
        for c in range(4):
            nc.tensor.matmul(
                pt[:, c * 512 : (c + 1) * 512],
                LT[:, T * 128 : (T + 1) * 128],
                RHS[:, c * 512 : (c + 1) * 512],
                start=True,
                stop=True,
            )
        TS = sbk.tile([128, N], F32, tag="TS")
        for c in range(4):
            cs = slice(c * 512, (c + 1) * 512)
            nc.scalar.copy(TS[:, cs], pt[:, cs])
        MK = sbk.tile([128, N], F32, tag="MK")
        nc.vector.scalar_tensor_tensor(MK[:], TS[:], BIGNEG, TS[:], OP.mult, OP.min)

        V1 = sbk.tile([128, 8], F32, tag="V1")
        nc.vector.max(V1[:], MK[:])
        nc.vector.max_index(rap(IU32, 0, 1, 128, T, [[16, 8]]), V1[:], MK[:])
        nc.vector.match_replace(MK[:], V1[:], MK[:], -3e38)
        V2 = sbk.tile([128, 8], F32, tag="V2")
        nc.vector.max(V2[:], MK[:])
        nc.vector.max_index(rap(IU32, 0, 1, 128, 128 + T, [[16, 8]]), V2[:], MK[:])

    IF32 = sb.tile([128, 256], F32, tag="IF32")
    nc.vector.tensor_copy(IF32[:], IU32[:])

    # ---------------- Phase C: index lists + gather ----------------
    IDX1 = sb.tile([128, 128], mybir.dt.int16, tag="IDX1")
    IDX2 = sb.tile([128, 128], mybir.dt.int16, tag="IDX2")
    pt1 = ps2.tile([128, 128], F32, tag="trpsum")
    nc.tensor.transpose(pt1[:], IF32[:, 0:128], ident[:])
    nc.vector.tensor_copy(IDX1[:], pt1[:])
    pt2 = ps2.tile([128, 128], F32, tag="trpsum")
    nc.tensor.transpose(pt2[:], IF32[:, 128:256], ident[:])
    nc.vector.tensor_copy(IDX2[0:32, :], pt2[0:32, :])
    nc.gpsimd.memset(IDX2[32:64, :], 0)
    nc.gpsimd.memset(IDX2[64:128, :], 0)

    G1 = sb.tile([128, N], F32, tag="G1")
    G2 = sb.tile([128, N], F32, tag="G2")
    nc.gpsimd.ap_gather(G1[:], D6[:], IDX1[:], channels=128, num_elems=N, d=1, num_idxs=N)
    nc.gpsimd.ap_gather(G2[:], D6[:], IDX2[:], channels=128, num_elems=N, d=1, num_idxs=N)
    # plain coordinate tables into G2 rows 32..37, free-permuted n->i order:
    # i = p*16 + T for n = T*128 + p
    for T in range(NB):
        nc.sync.dma_start(
            rap(G2, 32, 1, 6, T, [[16, 128]]),
            rap(D6, 0, 1, 6, T * 128, [[1, 128]]),
        )

    # ---------------- Phase D: transpose gathered data to point layout -----
    GN = sb.tile([128, NB, K, 6], F32, tag="GN")
    XP = sb.tile([128, NB, 6], F32, tag="XP")
    for s in range(NB):
        q1 = ps2.tile([128, 128], F32, tag="trpsum")
        nc.tensor.transpose(q1[:], G1[:, s * 128 : (s + 1) * 128], ident[:])
        nc.scalar.copy(GN[:, s, 0:8, :], rap(q1, 0, 1, 128, 0, [[16, 8], [1, 6]]))
        q2 = ps2.tile([128, 40], F32, tag="trpsum")
        nc.tensor.transpose(
            q2[:, 0:38], G2[0:38, s * 128 : (s + 1) * 128], ident[0:38, 0:38]
        )
        nc.vector.tensor_copy(GN[:, s, 8:10, :], rap(q2, 0, 1, 128, 0, [[16, 2], [1, 6]]))
        nc.vector.tensor_copy(XP[:, s, :], rap(q2, 0, 1, 128, 32, [[1, 6]]))

    # ---------------- Phase E: triangles + loss ----------------
    DK = sb.tile([128, NB, K, 6], F32, tag="DK")
    xp_b = rap(XP, 0, 1, 128, 0, [[6, NB], [0, K], [1, 6]])
    nc.vector.tensor_tensor(DK[:], xp_b, GN[:], OP.subtract)
    nc.scalar.square(DK[:], DK[:])
    EK = sb.tile([128, NB, K, 2], F32, tag="EK")
    nc.vector.tensor_reduce(
        EK[:], DK[:].rearrange("p s k (t c) -> p (s k t) c", c=3), AX.X, OP.add
    )

    # ETRI[p, s, j, st, e]; e = (d01, d12, d02), st = (src, tgt)
    ETRI = sb.tile([128, NB, NPAIR, 2, 3], F32, tag="ETRI")
    joff = 0
    for a in range(K - 1):
        nrep = K - 1 - a
        nc.scalar.copy(
            rap(ETRI, 0, 1, 128, joff * 6 + 0, [[NPAIR * 6, NB], [6, nrep], [3, 2]]),
            rap(EK, 0, 1, 128, a * 2, [[2 * K, NB], [0, nrep], [1, 2]]),
        )
        nc.scalar.copy(
            rap(ETRI, 0, 1, 128, joff * 6 + 2, [[NPAIR * 6, NB], [6, nrep], [3, 2]]),
            rap(EK, 0, 1, 128, (a + 1) * 2, [[2 * K, NB], [2, nrep], [1, 2]]),
        )
        joff += nrep

    DAB = sb.tile([128, NB, NPAIR, 6], F32, tag="DAB")
    joff = 0
    for a in range(K - 1):
        nrep = K - 1 - a
        nc.vector.tensor_tensor(
            rap(DAB, 0, 1, 128, joff * 6, [[NPAIR * 6, NB], [6, nrep], [1, 6]]),
            rap(GN, 0, 1, 128, a * 6, [[K * 6, NB], [0, nrep], [1, 6]]),
            rap(GN, 0, 1, 128, (a + 1) * 6, [[K * 6, NB], [6, nrep], [1, 6]]),
            OP.subtract,
        )
        joff += nrep
    nc.scalar.square(DAB[:], DAB[:])
    e12_dst = rap(ETRI, 0, 1, 128, 1, [[6, NB * NPAIR], [3, 2]])
    nc.vector.tensor_reduce(
        e12_dst, DAB[:].rearrange("p s j (t c) -> p (s j t) c", c=3), AX.X, OP.add
    )

    # EPS on tgt lengths
    tsl = rap(ETRI, 0, 1, 128, 3, [[6, NB * NPAIR], [1, 3]])
    CEPS = sb.tile([128, 1], F32, tag="CEPS")
    nc.vector.memset(CEPS[:], EPS)
    nc.scalar.activation(tsl, tsl, mybir.ActivationFunctionType.Identity, bias=CEPS[:])

    # sort3 (both st at once); planes strided by 3
    def eplane(t, e):
        return rap(t, 0, 1, 128, e, [[3, NB * NPAIR * 2]])

    SRT = sb.tile([128, NB, NPAIR, 2, 3], F32, tag="SRT")
    TL1 = sb.tile([128, NB, NPAIR, 2], F32, tag="TL1")
    TH1 = sb.tile([128, NB, NPAIR, 2], F32, tag="TH1")
    e0, e1, e2 = eplane(ETRI, 0), eplane(ETRI, 1), eplane(ETRI, 2)
    s0, s1, s2 = eplane(SRT, 0), eplane(SRT, 1), eplane(SRT, 2)
    nc.vector.tensor_tensor(TL1[:], e0, e1, OP.min)
    nc.vector.tensor_tensor(TH1[:], e0, e1, OP.max)
    nc.vector.tensor_tensor(s0, TL1[:], e2, OP.min)
    nc.vector.tensor_tensor(TL1[:], TL1[:], e2, OP.max)
    nc.vector.tensor_tensor(s1, TH1[:], TL1[:], OP.min)
    nc.vector.tensor_tensor(s2, TH1[:], TL1[:], OP.max)

    # num/den
    S_s = rap(SRT, 0, 1, 128, 0, [[6, NB * NPAIR], [1, 3]])
    S_t = rap(SRT, 0, 1, 128, 3, [[6, NB * NPAIR], [1, 3]])
    DN = sb.tile([128, NB, NPAIR, 3], F32, tag="DN")
    SM = sb.tile([128, NB, NPAIR, 3], F32, tag="SM")
    nc.vector.tensor_tensor(DN[:], S_s, S_t, OP.subtract)
    nc.vector.tensor_tensor(SM[:], S_s, S_t, OP.add)
    nc.scalar.square(DN[:], DN[:])
    nc.scalar.square(SM[:], SM[:])
    NUM = sb.tile([128, NB, NPAIR], F32, tag="NUM")
    DEN = sb.tile([128, NB, NPAIR], F32, tag="DEN")
    nc.vector.tensor_reduce(NUM[:], DN[:].rearrange("p s j c -> p (s j) c"), AX.X, OP.add)
    nc.vector.tensor_reduce(DEN[:], SM[:].rearrange("p s j c -> p (s j) c"), AX.X, OP.add)
    NEG = NUM
    nc.vector.reciprocal(DEN[:], DEN[:])
    nc.vector.scalar_tensor_tensor(NEG[:], NUM[:], -1.0, DEN[:], OP.mult, OP.mult)

    # top-10 smallest of 45 per (p, s)
    LV1 = sb.tile([128, NB, 8], F32, tag="LV1")
    LV2 = sb.tile([128, NB, 8], F32, tag="LV2")
    for s in range(NB):
        nc.vector.max(LV1[:, s, :], NEG[:, s, :])
        nc.vector.match_replace(NEG[:, s, :], LV1[:, s, :], NEG[:, s, :], BIGNEG)
        nc.vector.max(LV2[:, s, :], NEG[:, s, :])

    V10 = sb.tile([128, NB, 10], F32, tag="V10")
    nc.scalar.copy(V10[:, :, 0:8], LV1[:])
    nc.scalar.copy(V10[:, :, 8:10], LV2[:, :, 0:2])
    LX = sb.tile([128, NB, 10], F32, tag="LX")
    nc.vector.tensor_scalar(LX[:], V10[:], -1.0, EPS, OP.mult, OP.add)
    Y = sb.tile([128, NB, 10], F32, tag="Y")
    nc.scalar.activation(Y[:], LX[:], mybir.ActivationFunctionType.Sqrt)
    Q = sb.tile([128, NB, 10], F32, tag="Q")
    for _ in range(2):
        nc.vector.reciprocal(Q[:], Y[:])
        nc.vector.tensor_tensor(Q[:], LX[:], Q[:], OP.mult)
        nc.vector.tensor_tensor(Q[:], Y[:], Q[:], OP.add)
        nc.vector.tensor_scalar(Y[:], Q[:], 0.5, None, OP.mult)

    SUM10 = sb.tile([128, NB], F32, tag="SUM10")
    nc.vector.tensor_reduce(SUM10[:], Y[:], AX.X, OP.add)
    LOSS = sb.tile([128, NB], F32, tag="LOSS")
    nc.vector.tensor_scalar(LOSS[:], SUM10[:], 0.1, None, OP.mult)

    # batch min
    M1 = sb.tile([128, 1], F32, tag="M1")
    nc.vector.tensor_reduce(M1[:], LOSS[:], AX.X, OP.min)
    ptm = ps2.tile([1, 128], F32, tag="trpsum")
    nc.tensor.transpose(ptm[:], M1[:], ident[:])
    MR = sb.tile([1, 128], F32, tag="MR")
    nc.vector.tensor_copy(MR[:], ptm[:])
    MC = sb.tile([1, 1], F32, tag="MC")
    nc.vector.tensor_reduce(MC[:], MR[:], AX.X, OP.min)
    MB = sb.tile([128, 1], F32, tag="MB")
    nc.gpsimd.partition_broadcast(MB[:], MC[:])

    W = sb.tile([128, NB], F32, tag="W")
    nc.vector.tensor_scalar(W[:], LOSS[:], MB[:], CTH, OP.subtract, OP.is_lt)

    # out: transpose W -> WT[s, pi], then one DMA
    # n = (pi%16)*128 + 8*s + pi//16 with pi = 16m + T
    ptw = ps2.tile([16, 128], F32, tag="trpsum")
    nc.tensor.transpose(ptw[:], W[:], ident[:])
    WT = sb.tile([16, 128], F32, tag="WT")
    nc.scalar.copy(WT[:], ptw[:])
    src_ap = rap(WT, 0, 1, 16, 0, [[16, 8], [1, 16]])
    dst_ap = bass.AP(
        tensor=out_d.tensor,
        offset=out_d[b].offset,
        ap=[[8, 16], [1, 8], [128, 16]],
    )
    nc.sync.dma_start(dst_ap, src_ap)


def build_program():
    if "nc" in _CACHE:
        return _CACHE["nc"]
    nc = bacc.Bacc(
        "TRN2",
        target_bir_lowering=False,
        debug=False,
        enable_asserts=False,
        num_devices=8,
    )
    src_d = nc.dram_tensor("src", [BPC, 3, N], F32, kind="ExternalInput").ap()
    tgt_d = nc.dram_tensor("tgt", [BPC, 3, N], F32, kind="ExternalInput").ap()
    out_d = nc.dram_tensor("out", [BPC, N], F32, kind="ExternalOutput").ap()

    with tile.TileContext(nc) as tc, ExitStack() as ctx:
        sb = ctx.enter_context(tc.tile_pool(name="sb", bufs=1))
        sbk = ctx.enter_context(tc.tile_pool(name="sbk", bufs=2))
        ps1 = ctx.enter_context(tc.tile_pool(name="ps1", bufs=1, space="PSUM"))
        ps2 = ctx.enter_context(tc.tile_pool(name="ps2", bufs=4, space="PSUM"))
        pools = {"sb": sb, "sbk": sbk, "ps1": ps1, "ps2": ps2}
        ident = sb.tile([128, 128], F32, tag="ident")
        masks.make_identity(nc, ident[:])
        sts = [
            _build_setup(ctx, tc, pools, b, src_d, tgt_d, ident) for b in range(BPC)
        ]
        for b in range(BPC):
            _build_main(ctx, tc, pools, b, sts[b], out_d, ident)

    nc.compile()
    _CACHE["nc"] = nc
    return nc


def kernel(**inputs):
    src = np.ascontiguousarray(np.asarray(inputs["src"], dtype=np.float32))
    tgt = np.ascontiguousarray(np.asarray(inputs["tgt"], dtype=np.float32))
    B = src.shape[0]
    ncores = 8
    bpc = B // ncores
    nc = build_program()
    in_maps = [
        {"src": src[i * bpc : (i + 1) * bpc], "tgt": tgt[i * bpc : (i + 1) * bpc]}
        for i in range(ncores)
    ]
    res = run_bass_kernel_spmd(nc, in_maps, core_ids=list(range(ncores)))
    return np.concatenate([res.results[i]["out"] for i in range(ncores)], axis=0)



# revision 9
# speedup vs baseline: 1.5829x; 1.5829x over previous
"""Trainium2 Bass kernel for nn_DIT_11458972746143 (retrieval_knn).

B=16 batches sharded over 8 NeuronCores (2 per core). v2 design:
  1. KNN distance: PE fp16 hi/lo-split matmul (15 contraction rows) computes
     t'' = d2 - C0 in PSUM at 1 cyc/row (4x faster than fp32).
  2. Packed keys: one DVE scalar_tensor_tensor does
     key = (bits(t'') & 0xFFFFFF00) ^ (0x7FFFFF00 | (255 - j%256)).
     Valid candidates (t''>0) become positive floats descending in d2 with the
     segment-local column index in the low byte; masked entries (d2<C0, incl.
     self) keep the sign bit and sort below everything. No mask pass, no
     match_replace, no full-width max_index.
  3. Top-10: 8 segmented max8 (256 wide) -> 64 candidates; small max8 rounds +
     max_index on the candidate tile give the top-10 with positions; the
     segment comes from position via a float floor trick, the local index from
     the key's low byte. Validated bit-exact vs reference on the fixed input.
  4. Phases C/D/E (gathers, triangles, loss, threshold) as in v1.
"""

from contextlib import ExitStack

import numpy as np

import concourse.bass as bass
import concourse.tile as tile
from concourse import bacc, masks, mybir
from concourse.bass_utils import run_bass_kernel_spmd

F32 = mybir.dt.float32
F16 = mybir.dt.float16
U32 = mybir.dt.uint32
I32 = mybir.dt.int32
U16 = mybir.dt.uint16
OP = mybir.AluOpType
AX = mybir.AxisListType
AF = mybir.ActivationFunctionType

N = 2048
NB = 16            # row blocks of 128
BPC = 2            # batches per core
K = 10
NPAIR = 45
BIGNEG = -1e30
C0 = float(np.float32(np.float64(0.1) - np.float64(1e-7)))
CTH = float(np.float32(np.log(np.float64(7.0) / 3.0) / 30.0))
EPS = 1e-6
C0H = float(np.float16(-C0))
C0L1024 = float(np.float16(1024.0 * (-C0 - float(np.float16(-C0)))))
MAGIC = float(np.float32(2.0 ** 23))

_CACHE = {}


def rap(t, p_start, p_step, p_count, free_off, free_dims):
    """Raw AP over tile t: partitions [p_start::p_step] x free pattern."""
    base = t[:]
    pitch = base.ap[0][0]
    return bass.AP(
        tensor=t.tensor,
        offset=base.offset + p_start * pitch + free_off,
        ap=[[p_step * pitch, p_count]] + list(free_dims),
    )


def bcast(t, count):
    """[128,1] scalar tile broadcast to [128, count] via stride-0 free dim."""
    base = t[:]
    return bass.AP(tensor=t.tensor, offset=base.offset,
                   ap=[[base.ap[0][0], base.ap[0][1]], [0, count]])


def _build_consts(tc, sb):
    nc = tc.nc
    st = {}
    # XT2048[j] = 0x7FFFFFFF - (j % 256), as int32
    XT = sb.tile([128, N], I32, tag="XT")
    nc.gpsimd.iota(XT[:], [[0, 8], [-1, 256]], base=255, channel_multiplier=0)
    C7F00 = sb.tile([128, 1], I32, tag="C7F00")
    nc.gpsimd.iota(C7F00[:], [[0, 1]], base=0x7FFFFF00, channel_multiplier=0)
    nc.vector.tensor_tensor(XT[:], XT[:], bcast(C7F00, N), OP.bitwise_xor)
    st["XT"] = XT
    MHI = sb.tile([128, 1], I32, tag="MHI")
    nc.gpsimd.iota(MHI[:], [[0, 1]], base=-256, channel_multiplier=0)
    st["MHI"] = MHI
    C255 = sb.tile([128, 1], I32, tag="C255")
    nc.gpsimd.iota(C255[:], [[0, 1]], base=255, channel_multiplier=0)
    st["C255"] = C255
    C255T = sb.tile([128, 16], I32, tag="C255T")
    nc.gpsimd.iota(C255T[:], [[0, 16]], base=255, channel_multiplier=0)
    st["C255T"] = C255T
    BOFF = sb.tile([128, 1], F32, tag="BOFF")
    nc.vector.memset(BOFF[:], 0.5625)
    st["BOFF"] = BOFF
    BMAG = sb.tile([128, 1], F32, tag="BMAG")
    nc.vector.memset(BMAG[:], MAGIC)
    st["BMAG"] = BMAG
    BNEG = sb.tile([128, 1], F32, tag="BNEG")
    nc.vector.memset(BNEG[:], -(MAGIC + 1.0))
    st["BNEG"] = BNEG
    return st


def _build_setup(ctx, tc, pools, b, src_d, tgt_d, ident):
    """Phase A: per-batch fp16-split matmul operand prep + gather tables."""
    nc = tc.nc
    sb = pools["sb"]

    X = sb.tile([3, N], F32, tag="DAB")
    nc.sync.dma_start(X[:], src_d[b])
    # 48-layout: X48[p, d*16 + t] = X[d, p*16 + t]   (point n = 16p + t)
    X48 = sb.tile([128, 48], F32, tag=f"X48{b}")
    for d in range(3):
        nc.sync.dma_start(
            rap(X48, 0, 1, 128, d * 16, [[1, 16]]),
            rap(X, d, 1, 1, 0, [[16, 128], [1, 16]]),
        )
    H48 = sb.tile([128, 48], F16, tag=f"H48{b}")
    nc.vector.tensor_copy(H48[:], X48[:])
    HF48 = sb.tile([128, 48], F32, tag=f"HF48{b}")
    nc.vector.tensor_copy(HF48[:], H48[:])
    L48 = sb.tile([128, 48], F32, tag=f"L48{b}")
    nc.vector.tensor_tensor(L48[:], X48[:], HF48[:], OP.subtract)
    # fp16 variants
    H16d = sb.tile([128, 48], F16, tag=f"H16d{b}")      # h / 1024
    nc.vector.tensor_scalar(H16d[:], HF48[:], 1.0 / 1024.0, None, OP.mult)
    Hm2 = sb.tile([128, 48], F16, tag=f"Hm2{b}")        # -2h
    nc.vector.tensor_scalar(Hm2[:], HF48[:], -2.0, None, OP.mult)
    Hm512 = sb.tile([128, 48], F16, tag=f"Hm512{b}")    # -h/512
    nc.vector.tensor_scalar(Hm512[:], HF48[:], -1.0 / 512.0, None, OP.mult)
    L1024 = sb.tile([128, 48], F16, tag=f"L1024{b}")    # 1024 l
    nc.vector.tensor_scalar(L1024[:], L48[:], 1024.0, None, OP.mult)
    Lm2048 = sb.tile([128, 48], F16, tag=f"Lm2048{b}")  # -2048 l
    nc.vector.tensor_scalar(Lm2048[:], L48[:], -2048.0, None, OP.mult)
    # squares and per-point |x|^2
    XSQ = sb.tile([128, 48], F32, tag=f"XSQ{b}")
    nc.scalar.square(XSQ[:], X48[:])
    SQ16 = sb.tile([128, 16], F32, tag=f"SQ16{b}")
    nc.vector.tensor_reduce(
        SQ16[:], rap(XSQ, 0, 1, 128, 0, [[1, 16], [16, 3]]), AX.X, OP.add
    )
    SQH = sb.tile([128, 16], F16, tag=f"SQH{b}")
    nc.vector.tensor_copy(SQH[:], SQ16[:])
    SQHF = sb.tile([128, 16], F32, tag=f"SQHF{b}")
    nc.vector.tensor_copy(SQHF[:], SQH[:])
    SQL = sb.tile([128, 16], F32, tag=f"SQL{b}")
    nc.vector.tensor_tensor(SQL[:], SQ16[:], SQHF[:], OP.subtract)
    SQLS = sb.tile([128, 16], F16, tag=f"SQLS{b}")      # 1024 * sql
    nc.vector.tensor_scalar(SQLS[:], SQL[:], 1024.0, None, OP.mult)

    # assemble LT16 (stationary) / RHS16 (moving), [15, 2048] fp16
    LT = sb.tile([15, N], F16, tag=f"LT{b}")
    RHS = sb.tile([15, N], F16, tag=f"RHS{b}")

    def row_from48(dst, k, tile48, off):
        nc.sync.dma_start(
            rap(dst, k, 1, 1, 0, [[16, 128], [1, 16]]),
            rap(tile48, 0, 1, 128, off, [[1, 16]]),
        )

    for d in range(3):
        row_from48(LT, 0 + d, H48, d * 16)
        row_from48(LT, 3 + d, H16d, d * 16)
        row_from48(LT, 6 + d, L1024, d * 16)
        row_from48(RHS, 0 + d, Hm2, d * 16)
        row_from48(RHS, 3 + d, Lm2048, d * 16)
        row_from48(RHS, 6 + d, Hm512, d * 16)
    row_from48(LT, 9, SQH, 0)
    row_from48(LT, 10, SQLS, 0)
    row_from48(RHS, 11, SQH, 0)
    row_from48(RHS, 12, SQLS, 0)
    CR = sb.tile([1, N], F16, tag=f"CR{b}")
    nc.vector.memset(CR[:], 1.0)
    nc.sync.dma_start(LT[11:12, :], CR[:])
    nc.sync.dma_start(LT[13:14, :], CR[:])
    nc.sync.dma_start(RHS[9:10, :], CR[:])
    nc.vector.memset(CR[:], 1.0 / 1024.0)
    nc.sync.dma_start(LT[12:13, :], CR[:])
    nc.sync.dma_start(LT[14:15, :], CR[:])
    nc.sync.dma_start(RHS[10:11, :], CR[:])
    nc.vector.memset(CR[:], C0H)
    nc.sync.dma_start(RHS[13:14, :], CR[:])
    nc.vector.memset(CR[:], C0L1024)
    nc.sync.dma_start(RHS[14:15, :], CR[:])

    # gather table: rows 16g+c, c in 0..5 = (sx, sy, sz, tx, ty, tz)
    D6 = sb.tile([128, N], F32, tag=f"D6{b}")
    nc.gpsimd.memset(D6[:], 0.0)
    for g in range(8):
        nc.sync.dma_start(D6[16 * g : 16 * g + 3, :], src_d[b])
        nc.sync.dma_start(D6[16 * g + 3 : 16 * g + 6, :], tgt_d[b])
    return {"LT": LT, "RHS": RHS, "D6": D6}


def _build_main(ctx, tc, pools, b, st, cst, out_d, ident):
    nc = tc.nc
    sb, sbk = pools["sb"], pools["sbk"]
    ps1, ps2 = pools["ps1"], pools["ps2"]
    LT, RHS, D6 = st["LT"], st["RHS"], st["D6"]
    XT, MHI, C255, C255T = cst["XT"], cst["MHI"], cst["C255"], cst["C255T"]

    # ---------------- Phase B: KNN per block (packed keys) ----------------
    IU32 = sb.tile([128, 256], U32, tag="IU32")  # slots k*16 + T
    for T in range(NB):
        KEY = sbk.tile([128, N], F32, tag="KEY")
        for h in range(2):
            pt = ps1.tile([128, N // 2], F32, tag="knnpsum")
            for c in range(2):
                nc.tensor.matmul(
                    pt[:, c * 512 : (c + 1) * 512],
                    LT[:, T * 128 : (T + 1) * 128],
                    RHS[:, h * 1024 + c * 512 : h * 1024 + (c + 1) * 512],
                    start=True,
                    stop=True,
                )
            # key = (bits(t'') & 0xFFFFFF00) ^ XT
            nc.vector.scalar_tensor_tensor(
                KEY[:, h * 1024 : (h + 1) * 1024].bitcast(I32),
                pt[:].bitcast(I32),
                MHI[:],
                XT[:, h * 1024 : (h + 1) * 1024],
                OP.bitwise_and,
                OP.bitwise_xor,
            )
        CAND = sbk.tile([128, 64], F32, tag="CAND")
        for s in range(8):
            nc.vector.max(CAND[:, s * 8 : (s + 1) * 8], KEY[:, s * 256 : (s + 1) * 256])
        W16T = sbk.tile([128, 16], F32, tag="W16T")
        nc.vector.max(W16T[:, 0:8], CAND[:])
        CANDR = sbk.tile([128, 64], F32, tag="CANDR")
        nc.vector.match_replace(CANDR[:], W16T[:, 0:8], CAND[:], -3e38)
        nc.vector.max(W16T[:, 8:16], CANDR[:])
        POS = sbk.tile([128, 16], U32, tag="POS")
        nc.vector.max_index(POS[:, 0:8], W16T[:, 0:8], CAND[:])
        nc.vector.max_index(POS[:, 8:16], W16T[:, 8:16], CAND[:])
        # decode: loc = (key & 0xFF) ^ 0xFF ; seg = floor(pos/8) ; idx = 256*seg + loc
        LOCI = sbk.tile([128, 16], I32, tag="LOCI")
        nc.vector.scalar_tensor_tensor(
            LOCI[:], W16T[:].bitcast(I32), C255[:], C255T[:],
            OP.bitwise_and, OP.bitwise_xor,
        )
        LOCF = sbk.tile([128, 16], F32, tag="LOCF")
        nc.vector.tensor_copy(LOCF[:], LOCI[:])
        POSF = sbk.tile([128, 16], F32, tag="POSF")
        nc.vector.tensor_copy(POSF[:], POS[:])
        SEGF = sbk.tile([128, 16], F32, tag="SEGF")
        nc.scalar.activation(SEGF[:], POSF[:], AF.Identity,
                             bias=cst["BOFF"][:], scale=0.125)
        nc.scalar.activation(SEGF[:], SEGF[:], AF.Identity, bias=cst["BMAG"][:])
        nc.scalar.activation(SEGF[:], SEGF[:], AF.Identity, bias=cst["BNEG"][:])
        IDXF = sbk.tile([128, 16], F32, tag="IDXF")
        nc.vector.scalar_tensor_tensor(
            IDXF[:], SEGF[:], 256.0, LOCF[:], OP.mult, OP.add
        )
        nc.vector.tensor_copy(rap(IU32, 0, 1, 128, T, [[16, 10]]), IDXF[:, 0:10])

    IF32 = sb.tile([128, 256], F32, tag="IF32")
    nc.vector.tensor_copy(IF32[:], IU32[:])

    # ---------------- Phase C: index lists + gather ----------------
    IDX1 = sb.tile([128, 128], mybir.dt.int16, tag="IDX1")
    IDX2 = sb.tile([128, 128], mybir.dt.int16, tag="IDX2")
    pt1 = ps2.tile([128, 128], F32, tag="trpsum")
    nc.tensor.transpose(pt1[:], IF32[:, 0:128], ident[:])
    nc.vector.tensor_copy(IDX1[:], pt1[:])
    pt2 = ps2.tile([128, 128], F32, tag="trpsum")
    nc.tensor.transpose(pt2[:], IF32[:, 128:256], ident[:])
    nc.vector.tensor_copy(IDX2[0:32, :], pt2[0:32, :])
    nc.gpsimd.memset(IDX2[32:64, :], 0)
    nc.gpsimd.memset(IDX2[64:128, :], 0)

    G1 = sb.tile([128, N], F32, tag="G1")
    G2 = sb.tile([128, N], F32, tag="G2")
    nc.gpsimd.ap_gather(G1[:], D6[:], IDX1[:], channels=128, num_elems=N, d=1, num_idxs=N)
    nc.gpsimd.ap_gather(G2[:], D6[:], IDX2[:], channels=128, num_elems=N, d=1, num_idxs=N)
    # plain coordinate tables into G2 rows 32..37, free-permuted n->i order:
    # i = p*16 + T for point n = T*128 + p
    for T in range(NB):
        nc.sync.dma_start(
            rap(G2, 32, 1, 6, T, [[16, 128]]),
            rap(D6, 0, 1, 6, T * 128, [[1, 128]]),
        )

    # ---------------- Phase D: transpose gathered data to point layout -----
    GN = sb.tile([128, NB, K, 6], F32, tag="GN")
    XP = sb.tile([128, NB, 6], F32, tag="XP")
    for s in range(NB):
        q1 = ps2.tile([128, 128], F32, tag="trpsum")
        nc.tensor.transpose(q1[:], G1[:, s * 128 : (s + 1) * 128], ident[:])
        nc.scalar.copy(GN[:, s, 0:8, :], rap(q1, 0, 1, 128, 0, [[16, 8], [1, 6]]))
        q2 = ps2.tile([128, 40], F32, tag="trpsum")
        nc.tensor.transpose(
            q2[:, 0:38], G2[0:38, s * 128 : (s + 1) * 128], ident[0:38, 0:38]
        )
        nc.vector.tensor_copy(GN[:, s, 8:10, :], rap(q2, 0, 1, 128, 0, [[16, 2], [1, 6]]))
        nc.vector.tensor_copy(XP[:, s, :], rap(q2, 0, 1, 128, 32, [[1, 6]]))

    # ---------------- Phase E: triangles + loss ----------------
    DK = sb.tile([128, NB, K, 6], F32, tag="DK")
    xp_b = rap(XP, 0, 1, 128, 0, [[6, NB], [0, K], [1, 6]])
    nc.vector.tensor_tensor(DK[:], xp_b, GN[:], OP.subtract)
    nc.scalar.square(DK[:], DK[:])
    EK = sb.tile([128, NB, K, 2], F32, tag="EK")
    nc.vector.tensor_reduce(
        EK[:], DK[:].rearrange("p s k (t c) -> p (s k t) c", c=3), AX.X, OP.add
    )

    # ETRI[p, s, j, st, e]; e = (d01, d12, d02), st = (src, tgt)
    ETRI = sb.tile([128, NB, NPAIR, 2, 3], F32, tag="ETRI")
    joff = 0
    for a in range(K - 1):
        nrep = K - 1 - a
        nc.scalar.copy(
            rap(ETRI, 0, 1, 128, joff * 6 + 0, [[NPAIR * 6, NB], [6, nrep], [3, 2]]),
            rap(EK, 0, 1, 128, a * 2, [[2 * K, NB], [0, nrep], [1, 2]]),
        )
        nc.scalar.copy(
            rap(ETRI, 0, 1, 128, joff * 6 + 2, [[NPAIR * 6, NB], [6, nrep], [3, 2]]),
            rap(EK, 0, 1, 128, (a + 1) * 2, [[2 * K, NB], [2, nrep], [1, 2]]),
        )
        joff += nrep

    DAB = sb.tile([128, NB, NPAIR, 6], F32, tag="DAB")
    joff = 0
    for a in range(K - 1):
        nrep = K - 1 - a
        nc.vector.tensor_tensor(
            rap(DAB, 0, 1, 128, joff * 6, [[NPAIR * 6, NB], [6, nrep], [1, 6]]),
            rap(GN, 0, 1, 128, a * 6, [[K * 6, NB], [0, nrep], [1, 6]]),
            rap(GN, 0, 1, 128, (a + 1) * 6, [[K * 6, NB], [6, nrep], [1, 6]]),
            OP.subtract,
        )
        joff += nrep
    nc.scalar.square(DAB[:], DAB[:])
    e12_dst = rap(ETRI, 0, 1, 128, 1, [[6, NB * NPAIR], [3, 2]])
    nc.vector.tensor_reduce(
        e12_dst, DAB[:].rearrange("p s j (t c) -> p (s j t) c", c=3), AX.X, OP.add
    )

    # EPS on tgt lengths
    tsl = rap(ETRI, 0, 1, 128, 3, [[6, NB * NPAIR], [1, 3]])
    CEPS = sb.tile([128, 1], F32, tag="CEPS")
    nc.vector.memset(CEPS[:], EPS)
    nc.scalar.activation(tsl, tsl, AF.Identity, bias=CEPS[:])

    # sort3 (both st at once); planes strided by 3
    def eplane(t, e):
        return rap(t, 0, 1, 128, e, [[3, NB * NPAIR * 2]])

    SRT = sb.tile([128, NB, NPAIR, 2, 3], F32, tag="SRT")
    TL1 = sb.tile([128, NB, NPAIR, 2], F32, tag="TL1")
    TH1 = sb.tile([128, NB, NPAIR, 2], F32, tag="TH1")
    e0, e1, e2 = eplane(ETRI, 0), eplane(ETRI, 1), eplane(ETRI, 2)
    s0, s1, s2 = eplane(SRT, 0), eplane(SRT, 1), eplane(SRT, 2)
    nc.vector.tensor_tensor(TL1[:], e0, e1, OP.min)
    nc.vector.tensor_tensor(TH1[:], e0, e1, OP.max)
    nc.vector.tensor_tensor(s0, TL1[:], e2, OP.min)
    nc.vector.tensor_tensor(TL1[:], TL1[:], e2, OP.max)
    nc.vector.tensor_tensor(s1, TH1[:], TL1[:], OP.min)
    nc.vector.tensor_tensor(s2, TH1[:], TL1[:], OP.max)

    # num/den
    S_s = rap(SRT, 0, 1, 128, 0, [[6, NB * NPAIR], [1, 3]])
    S_t = rap(SRT, 0, 1, 128, 3, [[6, NB * NPAIR], [1, 3]])
    DN = sb.tile([128, NB, NPAIR, 3], F32, tag="DN")
    SM = sb.tile([128, NB, NPAIR, 3], F32, tag="SM")
    nc.vector.tensor_tensor(DN[:], S_s, S_t, OP.subtract)
    nc.vector.tensor_tensor(SM[:], S_s, S_t, OP.add)
    nc.scalar.square(DN[:], DN[:])
    nc.scalar.square(SM[:], SM[:])
    NUM = sb.tile([128, NB, NPAIR], F32, tag="NUM")
    DEN = sb.tile([128, NB, NPAIR], F32, tag="DEN")
    nc.vector.tensor_reduce(NUM[:], DN[:].rearrange("p s j c -> p (s j) c"), AX.X, OP.add)
    nc.vector.tensor_reduce(DEN[:], SM[:].rearrange("p s j c -> p (s j) c"), AX.X, OP.add)
    NEG = NUM
    nc.vector.reciprocal(DEN[:], DEN[:])
    nc.vector.scalar_tensor_tensor(NEG[:], NUM[:], -1.0, DEN[:], OP.mult, OP.mult)

    # top-10 smallest of 45 per (p, s)
    LV1 = sb.tile([128, NB, 8], F32, tag="LV1")
    LV2 = sb.tile([128, NB, 8], F32, tag="LV2")
    for s in range(NB):
        nc.vector.max(LV1[:, s, :], NEG[:, s, :])
        nc.vector.match_replace(NEG[:, s, :], LV1[:, s, :], NEG[:, s, :], BIGNEG)
        nc.vector.max(LV2[:, s, :], NEG[:, s, :])

    V10 = sb.tile([128, NB, 10], F32, tag="V10")
    nc.scalar.copy(V10[:, :, 0:8], LV1[:])
    nc.scalar.copy(V10[:, :, 8:10], LV2[:, :, 0:2])
    LX = sb.tile([128, NB, 10], F32, tag="LX")
    nc.vector.tensor_scalar(LX[:], V10[:], -1.0, EPS, OP.mult, OP.add)
    Y = sb.tile([128, NB, 10], F32, tag="Y")
    nc.scalar.activation(Y[:], LX[:], AF.Sqrt)
    Q = sb.tile([128, NB, 10], F32, tag="Q")
    for _ in range(2):
        nc.vector.reciprocal(Q[:], Y[:])
        nc.vector.tensor_tensor(Q[:], LX[:], Q[:], OP.mult)
        nc.vector.tensor_tensor(Q[:], Y[:], Q[:], OP.add)
        nc.vector.tensor_scalar(Y[:], Q[:], 0.5, None, OP.mult)

    SUM10 = sb.tile([128, NB], F32, tag="SUM10")
    nc.vector.tensor_reduce(SUM10[:], Y[:], AX.X, OP.add)
    LOSS = sb.tile([128, NB], F32, tag="LOSS")
    nc.vector.tensor_scalar(LOSS[:], SUM10[:], 0.1, None, OP.mult)

    # batch min
    M1 = sb.tile([128, 1], F32, tag="M1")
    nc.vector.tensor_reduce(M1[:], LOSS[:], AX.X, OP.min)
    ptm = ps2.tile([1, 128], F32, tag="trpsum")
    nc.tensor.transpose(ptm[:], M1[:], ident[:])
    MR = sb.tile([1, 128], F32, tag="MR")
    nc.vector.tensor_copy(MR[:], ptm[:])
    MC = sb.tile([1, 1], F32, tag="MC")
    nc.vector.tensor_reduce(MC[:], MR[:], AX.X, OP.min)
    MB = sb.tile([128, 1], F32, tag="MB")
    nc.gpsimd.partition_broadcast(MB[:], MC[:])

    W = sb.tile([128, NB], F32, tag="W")
    nc.vector.tensor_scalar(W[:], LOSS[:], MB[:], CTH, OP.subtract, OP.is_lt)

    # out: transpose W -> WT[s, pi], then one DMA
    # n = (pi%16)*128 + 8*s + pi//16 with pi = 16m + T
    ptw = ps2.tile([16, 128], F32, tag="trpsum")
    nc.tensor.transpose(ptw[:], W[:], ident[:])
    WT = sb.tile([16, 128], F32, tag="WT")
    nc.scalar.copy(WT[:], ptw[:])
    src_ap = rap(WT, 0, 1, 16, 0, [[16, 8], [1, 16]])
    dst_ap = bass.AP(
        tensor=out_d.tensor,
        offset=out_d[b].offset,
        ap=[[8, 16], [1, 8], [128, 16]],
    )
    nc.sync.dma_start(dst_ap, src_ap)


def build_program():
    if "nc" in _CACHE:
        return _CACHE["nc"]
    nc = bacc.Bacc(
        "TRN2",
        target_bir_lowering=False,
        debug=False,
        enable_asserts=False,
        num_devices=8,
    )
    src_d = nc.dram_tensor("src", [BPC, 3, N], F32, kind="ExternalInput").ap()
    tgt_d = nc.dram_tensor("tgt", [BPC, 3, N], F32, kind="ExternalInput").ap()
    out_d = nc.dram_tensor("out", [BPC, N], F32, kind="ExternalOutput").ap()

    with tile.TileContext(nc) as tc, ExitStack() as ctx:
        sb = ctx.enter_context(tc.tile_pool(name="sb", bufs=1))
        sbk = ctx.enter_context(tc.tile_pool(name="sbk", bufs=2))
        ps1 = ctx.enter_context(tc.tile_pool(name="ps1", bufs=2, space="PSUM"))
        ps2 = ctx.enter_context(tc.tile_pool(name="ps2", bufs=4, space="PSUM"))
        pools = {"sb": sb, "sbk": sbk, "ps1": ps1, "ps2": ps2}
        ident = sb.tile([128, 128], F32, tag="ident")
        masks.make_identity(nc, ident[:])
        cst = _build_consts(tc, sb)
        sts = [
            _build_setup(ctx, tc, pools, b, src_d, tgt_d, ident) for b in range(BPC)
        ]
        for b in range(BPC):
            _build_main(ctx, tc, pools, b, sts[b], cst, out_d, ident)

    nc.compile()
    _CACHE["nc"] = nc
    return nc


def kernel(**inputs):
    src = np.ascontiguousarray(np.asarray(inputs["src"], dtype=np.float32))
    tgt = np.ascontiguousarray(np.asarray(inputs["tgt"], dtype=np.float32))
    B = src.shape[0]
    ncores = 8
    bpc = B // ncores
    nc = build_program()
    in_maps = [
        {"src": src[i * bpc : (i + 1) * bpc], "tgt": tgt[i * bpc : (i + 1) * bpc]}
        for i in range(ncores)
    ]
    res = run_bass_kernel_spmd(nc, in_maps, core_ids=list(range(ncores)))
    return np.concatenate([res.results[i]["out"] for i in range(ncores)], axis=0)


# revision 12
# speedup vs baseline: 1.6070x; 1.0152x over previous
"""Trainium2 Bass kernel for nn_DIT_11458972746143 (retrieval_knn).

B=16 batches sharded over 8 NeuronCores (2 per core). v2 design:
  1. KNN distance: PE fp16 hi/lo-split matmul (15 contraction rows) computes
     t'' = d2 - C0 in PSUM at 1 cyc/row (4x faster than fp32).
  2. Packed keys: one DVE scalar_tensor_tensor does
     key = (bits(t'') & 0xFFFFFF00) ^ (0x7FFFFF00 | (255 - j%256)).
     Valid candidates (t''>0) become positive floats descending in d2 with the
     segment-local column index in the low byte; masked entries (d2<C0, incl.
     self) keep the sign bit and sort below everything. No mask pass, no
     match_replace, no full-width max_index.
  3. Top-10: 8 segmented max8 (256 wide) -> 64 candidates; small max8 rounds +
     max_index on the candidate tile give the top-10 with positions; the
     segment comes from position via a float floor trick, the local index from
     the key's low byte. Validated bit-exact vs reference on the fixed input.
  4. Phases C/D/E (gathers, triangles, loss, threshold) as in v1.
"""

from contextlib import ExitStack

import numpy as np

import concourse.bass as bass
import concourse.tile as tile
from concourse import bacc, masks, mybir
from concourse.bass_utils import run_bass_kernel_spmd

F32 = mybir.dt.float32
F16 = mybir.dt.float16
U32 = mybir.dt.uint32
I32 = mybir.dt.int32
U16 = mybir.dt.uint16
OP = mybir.AluOpType
AX = mybir.AxisListType
AF = mybir.ActivationFunctionType

N = 2048
NB = 16            # row blocks of 128
BPC = 2            # batches per core
K = 10
NPAIR = 45
BIGNEG = -1e30
C0 = float(np.float32(np.float64(0.1) - np.float64(1e-7)))
CTH = float(np.float32(np.log(np.float64(7.0) / 3.0) / 30.0))
EPS = 1e-6
C0H = float(np.float16(-C0))
C0L1024 = float(np.float16(1024.0 * (-C0 - float(np.float16(-C0)))))
MAGIC = float(np.float32(2.0 ** 23))

_CACHE = {}


def rap(t, p_start, p_step, p_count, free_off, free_dims):
    """Raw AP over tile t: partitions [p_start::p_step] x free pattern."""
    base = t[:]
    pitch = base.ap[0][0]
    return bass.AP(
        tensor=t.tensor,
        offset=base.offset + p_start * pitch + free_off,
        ap=[[p_step * pitch, p_count]] + list(free_dims),
    )


def bcast(t, count):
    """[128,1] scalar tile broadcast to [128, count] via stride-0 free dim."""
    base = t[:]
    return bass.AP(tensor=t.tensor, offset=base.offset,
                   ap=[[base.ap[0][0], base.ap[0][1]], [0, count]])


def _build_consts(tc, sb):
    nc = tc.nc
    st = {}
    # XT2048[j] = 0x7FFFFFFF - (j % 256), as int32
    XT = sb.tile([128, N], I32, tag="XT")
    nc.gpsimd.iota(XT[:], [[0, 8], [-1, 256]], base=255, channel_multiplier=0)
    C7F00 = sb.tile([128, 1], I32, tag="C7F00")
    nc.gpsimd.iota(C7F00[:], [[0, 1]], base=0x7FFFFF00, channel_multiplier=0)
    nc.vector.tensor_tensor(XT[:], XT[:], bcast(C7F00, N), OP.bitwise_xor)
    st["XT"] = XT
    MHI = sb.tile([128, 1], I32, tag="MHI")
    nc.gpsimd.iota(MHI[:], [[0, 1]], base=-256, channel_multiplier=0)
    st["MHI"] = MHI
    C255 = sb.tile([128, 1], I32, tag="C255")
    nc.gpsimd.iota(C255[:], [[0, 1]], base=255, channel_multiplier=0)
    st["C255"] = C255
    C255T = sb.tile([128, 16], I32, tag="C255T")
    nc.gpsimd.iota(C255T[:], [[0, 16]], base=255, channel_multiplier=0)
    st["C255T"] = C255T
    BOFF = sb.tile([128, 1], F32, tag="BOFF")
    nc.vector.memset(BOFF[:], 0.5625)
    st["BOFF"] = BOFF
    BMAG = sb.tile([128, 1], F32, tag="BMAG")
    nc.vector.memset(BMAG[:], MAGIC)
    st["BMAG"] = BMAG
    BNEG = sb.tile([128, 1], F32, tag="BNEG")
    nc.vector.memset(BNEG[:], -(MAGIC + 1.0))
    st["BNEG"] = BNEG
    CR = sb.tile([4, N], F16, tag="CR")
    CS = sb.tile([1, N], F16, tag="CS")
    for i, v in enumerate([1.0, 1.0 / 1024.0, C0H, C0L1024]):
        nc.gpsimd.memset(CS[:], v)
        nc.sync.dma_start(CR[i : i + 1, :], CS[:])
    st["CR"] = CR
    return st


def _build_setup(ctx, tc, pools, b, src_d, tgt_d, ident, cst):
    """Phase A: per-batch fp16-split matmul operand prep + gather tables."""
    nc = tc.nc
    sb = pools["sb"]

    X = sb.tile([3, N], F32, tag="DAB")
    nc.sync.dma_start(X[:], src_d[b])
    # 48-layout: X48[p, d*16 + t] = X[d, p*16 + t]   (point n = 16p + t)
    X48 = sb.tile([128, 48], F32, tag=f"X48{b}")
    for d in range(3):
        nc.sync.dma_start(
            rap(X48, 0, 1, 128, d * 16, [[1, 16]]),
            rap(X, d, 1, 1, 0, [[16, 128], [1, 16]]),
        )
    H48 = sb.tile([128, 48], F16, tag=f"H48{b}")
    nc.vector.tensor_copy(H48[:], X48[:])
    HF48 = sb.tile([128, 48], F32, tag=f"HF48{b}")
    nc.vector.tensor_copy(HF48[:], H48[:])
    L48 = sb.tile([128, 48], F32, tag=f"L48{b}")
    nc.vector.tensor_tensor(L48[:], X48[:], HF48[:], OP.subtract)
    # fp16 variants
    H16d = sb.tile([128, 48], F16, tag=f"H16d{b}")      # h / 1024
    nc.vector.tensor_scalar(H16d[:], HF48[:], 1.0 / 1024.0, None, OP.mult)
    Hm2 = sb.tile([128, 48], F16, tag=f"Hm2{b}")        # -2h
    nc.vector.tensor_scalar(Hm2[:], HF48[:], -2.0, None, OP.mult)
    Hm512 = sb.tile([128, 48], F16, tag=f"Hm512{b}")    # -h/512
    nc.vector.tensor_scalar(Hm512[:], HF48[:], -1.0 / 512.0, None, OP.mult)
    L1024 = sb.tile([128, 48], F16, tag=f"L1024{b}")    # 1024 l
    nc.vector.tensor_scalar(L1024[:], L48[:], 1024.0, None, OP.mult)
    Lm2048 = sb.tile([128, 48], F16, tag=f"Lm2048{b}")  # -2048 l
    nc.vector.tensor_scalar(Lm2048[:], L48[:], -2048.0, None, OP.mult)
    # squares and per-point |x|^2
    XSQ = sb.tile([128, 48], F32, tag=f"XSQ{b}")
    nc.scalar.square(XSQ[:], X48[:])
    SQ16 = sb.tile([128, 16], F32, tag=f"SQ16{b}")
    nc.vector.tensor_reduce(
        SQ16[:], rap(XSQ, 0, 1, 128, 0, [[1, 16], [16, 3]]), AX.X, OP.add
    )
    SQH = sb.tile([128, 16], F16, tag=f"SQH{b}")
    nc.vector.tensor_copy(SQH[:], SQ16[:])
    SQHF = sb.tile([128, 16], F32, tag=f"SQHF{b}")
    nc.vector.tensor_copy(SQHF[:], SQH[:])
    SQL = sb.tile([128, 16], F32, tag=f"SQL{b}")
    nc.vector.tensor_tensor(SQL[:], SQ16[:], SQHF[:], OP.subtract)
    SQLS = sb.tile([128, 16], F16, tag=f"SQLS{b}")      # 1024 * sql
    nc.vector.tensor_scalar(SQLS[:], SQL[:], 1024.0, None, OP.mult)

    # assemble LT16 (stationary) / RHS16 (moving), [15, 2048] fp16
    LT = sb.tile([15, N], F16, tag=f"LT{b}")
    RHS = sb.tile([15, N], F16, tag=f"RHS{b}")

    def row_from48(dst, k, tile48, off):
        nc.sync.dma_start(
            rap(dst, k, 1, 1, 0, [[16, 128], [1, 16]]),
            rap(tile48, 0, 1, 128, off, [[1, 16]]),
        )

    for d in range(3):
        row_from48(LT, 0 + d, H48, d * 16)
        row_from48(LT, 3 + d, H16d, d * 16)
        row_from48(LT, 6 + d, L1024, d * 16)
        row_from48(RHS, 0 + d, Hm2, d * 16)
        row_from48(RHS, 3 + d, Lm2048, d * 16)
        row_from48(RHS, 6 + d, Hm512, d * 16)
    row_from48(LT, 9, SQH, 0)
    row_from48(LT, 10, SQLS, 0)
    row_from48(RHS, 11, SQH, 0)
    row_from48(RHS, 12, SQLS, 0)
    CR = cst["CR"]
    nc.sync.dma_start(LT[11:12, :], CR[0:1, :])
    nc.sync.dma_start(LT[13:14, :], CR[0:1, :])
    nc.sync.dma_start(RHS[9:10, :], CR[0:1, :])
    nc.sync.dma_start(LT[12:13, :], CR[1:2, :])
    nc.sync.dma_start(LT[14:15, :], CR[1:2, :])
    nc.sync.dma_start(RHS[10:11, :], CR[1:2, :])
    nc.sync.dma_start(RHS[13:14, :], CR[2:3, :])
    nc.sync.dma_start(RHS[14:15, :], CR[3:4, :])

    # gather table: rows 16g+c, c in 0..5 = (sx, sy, sz, tx, ty, tz)
    D6 = sb.tile([128, N], F32, tag=f"D6{b}")
    nc.gpsimd.memset(D6[:], 0.0)
    for g in range(8):
        nc.sync.dma_start(D6[16 * g : 16 * g + 3, :], src_d[b])
        nc.sync.dma_start(D6[16 * g + 3 : 16 * g + 6, :], tgt_d[b])
    return {"LT": LT, "RHS": RHS, "D6": D6}


def _build_main(ctx, tc, pools, b, st, cst, out_d, ident):
    nc = tc.nc
    sb, sbk = pools["sb"], pools["sbk"]
    ps1, ps2 = pools["ps1"], pools["ps2"]
    LT, RHS, D6 = st["LT"], st["RHS"], st["D6"]
    XT, MHI, C255, C255T = cst["XT"], cst["MHI"], cst["C255"], cst["C255T"]

    # ---------------- Phase B: KNN per block (packed keys) ----------------
    IU32 = sb.tile([128, 256], U32, tag="IU32")  # slots k*16 + T
    for T in range(NB):
        KEY = sbk.tile([128, N], F32, tag="KEY")
        for h in range(2):
            pt = ps1.tile([128, N // 2], F32, tag="knnpsum")
            for c in range(2):
                nc.tensor.matmul(
                    pt[:, c * 512 : (c + 1) * 512],
                    LT[:, T * 128 : (T + 1) * 128],
                    RHS[:, h * 1024 + c * 512 : h * 1024 + (c + 1) * 512],
                    start=True,
                    stop=True,
                )
            # key = (bits(t'') & 0xFFFFFF00) ^ XT
            nc.vector.scalar_tensor_tensor(
                KEY[:, h * 1024 : (h + 1) * 1024].bitcast(I32),
                pt[:].bitcast(I32),
                MHI[:],
                XT[:, h * 1024 : (h + 1) * 1024],
                OP.bitwise_and,
                OP.bitwise_xor,
            )
        CAND = sbk.tile([128, 64], F32, tag="CAND")
        for s in range(8):
            nc.vector.max(CAND[:, s * 8 : (s + 1) * 8], KEY[:, s * 256 : (s + 1) * 256])
        W16T = sbk.tile([128, 16], F32, tag="W16T")
        nc.vector.max(W16T[:, 0:8], CAND[:])
        CANDR = sbk.tile([128, 64], F32, tag="CANDR")
        nc.vector.match_replace(CANDR[:], W16T[:, 0:8], CAND[:], -3e38)
        nc.vector.max(W16T[:, 8:16], CANDR[:])
        POS = sbk.tile([128, 16], U32, tag="POS")
        nc.vector.max_index(POS[:, 0:8], W16T[:, 0:8], CAND[:])
        nc.vector.max_index(POS[:, 8:16], W16T[:, 8:16], CAND[:])
        # decode: loc = (key & 0xFF) ^ 0xFF ; seg = floor(pos/8) ; idx = 256*seg + loc
        LOCI = sbk.tile([128, 16], I32, tag="LOCI")
        nc.vector.scalar_tensor_tensor(
            LOCI[:], W16T[:].bitcast(I32), C255[:], C255T[:],
            OP.bitwise_and, OP.bitwise_xor,
        )
        LOCF = sbk.tile([128, 16], F32, tag="LOCF")
        nc.vector.tensor_copy(LOCF[:], LOCI[:])
        POSF = sbk.tile([128, 16], F32, tag="POSF")
        nc.vector.tensor_copy(POSF[:], POS[:])
        SEGF = sbk.tile([128, 16], F32, tag="SEGF")
        nc.scalar.activation(SEGF[:], POSF[:], AF.Identity,
                             bias=cst["BOFF"][:], scale=0.125)
        nc.scalar.activation(SEGF[:], SEGF[:], AF.Identity, bias=cst["BMAG"][:])
        nc.scalar.activation(SEGF[:], SEGF[:], AF.Identity, bias=cst["BNEG"][:])
        IDXF = sbk.tile([128, 16], F32, tag="IDXF")
        nc.vector.scalar_tensor_tensor(
            IDXF[:], SEGF[:], 256.0, LOCF[:], OP.mult, OP.add
        )
        nc.vector.tensor_copy(rap(IU32, 0, 1, 128, T, [[16, 10]]), IDXF[:, 0:10])

    IF32 = sb.tile([128, 256], F32, tag="IF32")
    nc.vector.tensor_copy(IF32[:], IU32[:])

    # ---------------- Phase C: index lists + gather ----------------
    IDX1 = sb.tile([128, 128], mybir.dt.int16, tag="IDX1")
    IDX2 = sb.tile([128, 128], mybir.dt.int16, tag="IDX2")
    pt1 = ps2.tile([128, 128], F32, tag="trpsum")
    nc.tensor.transpose(pt1[:], IF32[:, 0:128], ident[:])
    nc.vector.tensor_copy(IDX1[:], pt1[:])
    pt2 = ps2.tile([128, 128], F32, tag="trpsum")
    nc.tensor.transpose(pt2[:], IF32[:, 128:256], ident[:])
    nc.vector.tensor_copy(IDX2[0:32, :], pt2[0:32, :])
    nc.gpsimd.memset(IDX2[32:64, :], 0)
    nc.gpsimd.memset(IDX2[64:128, :], 0)

    G1 = sb.tile([128, N], F32, tag="G1")
    G2 = sb.tile([128, N], F32, tag="G2")
    nc.gpsimd.ap_gather(G1[:], D6[:], IDX1[:], channels=128, num_elems=N, d=1, num_idxs=N)
    nc.gpsimd.ap_gather(G2[:], D6[:], IDX2[:], channels=128, num_elems=N, d=1, num_idxs=N)
    # plain coordinate tables into G2 rows 32..37, free-permuted n->i order:
    # i = p*16 + T for point n = T*128 + p
    for T in range(NB):
        nc.sync.dma_start(
            rap(G2, 32, 1, 6, T, [[16, 128]]),
            rap(D6, 0, 1, 6, T * 128, [[1, 128]]),
        )

    # ---------------- Phase D: transpose gathered data to point layout -----
    GN = sb.tile([128, NB, K, 6], F32, tag="GN")
    XP = sb.tile([128, NB, 6], F32, tag="XP")
    for s in range(NB):
        q1 = ps2.tile([128, 128], F32, tag="trpsum")
        nc.tensor.transpose(q1[:], G1[:, s * 128 : (s + 1) * 128], ident[:])
        nc.scalar.copy(GN[:, s, 0:8, :], rap(q1, 0, 1, 128, 0, [[16, 8], [1, 6]]))
        q2 = ps2.tile([128, 40], F32, tag="trpsum")
        nc.tensor.transpose(
            q2[:, 0:38], G2[0:38, s * 128 : (s + 1) * 128], ident[0:38, 0:38]
        )
        nc.vector.tensor_copy(GN[:, s, 8:10, :], rap(q2, 0, 1, 128, 0, [[16, 2], [1, 6]]))
        nc.vector.tensor_copy(XP[:, s, :], rap(q2, 0, 1, 128, 32, [[1, 6]]))

    # ---------------- Phase E: triangles + loss ----------------
    DK = sb.tile([128, NB, K, 6], F32, tag="DK")
    xp_b = rap(XP, 0, 1, 128, 0, [[6, NB], [0, K], [1, 6]])
    nc.vector.tensor_tensor(DK[:], xp_b, GN[:], OP.subtract)
    nc.scalar.square(DK[:], DK[:])
    EK = sb.tile([128, NB, K, 2], F32, tag="EK")
    nc.vector.tensor_reduce(
        EK[:], DK[:].rearrange("p s k (t c) -> p (s k t) c", c=3), AX.X, OP.add
    )

    # ETRI[p, s, j, st, e]; e = (d01, d12, d02), st = (src, tgt)
    ETRI = sb.tile([128, NB, NPAIR, 2, 3], F32, tag="ETRI")
    joff = 0
    for a in range(K - 1):
        nrep = K - 1 - a
        nc.scalar.copy(
            rap(ETRI, 0, 1, 128, joff * 6 + 0, [[NPAIR * 6, NB], [6, nrep], [3, 2]]),
            rap(EK, 0, 1, 128, a * 2, [[2 * K, NB], [0, nrep], [1, 2]]),
        )
        nc.scalar.copy(
            rap(ETRI, 0, 1, 128, joff * 6 + 2, [[NPAIR * 6, NB], [6, nrep], [3, 2]]),
            rap(EK, 0, 1, 128, (a + 1) * 2, [[2 * K, NB], [2, nrep], [1, 2]]),
        )
        joff += nrep

    DAB = sb.tile([128, NB, NPAIR, 6], F32, tag="DAB")
    joff = 0
    for a in range(K - 1):
        nrep = K - 1 - a
        nc.vector.tensor_tensor(
            rap(DAB, 0, 1, 128, joff * 6, [[NPAIR * 6, NB], [6, nrep], [1, 6]]),
            rap(GN, 0, 1, 128, a * 6, [[K * 6, NB], [0, nrep], [1, 6]]),
            rap(GN, 0, 1, 128, (a + 1) * 6, [[K * 6, NB], [6, nrep], [1, 6]]),
            OP.subtract,
        )
        joff += nrep
    nc.scalar.square(DAB[:], DAB[:])
    e12_dst = rap(ETRI, 0, 1, 128, 1, [[6, NB * NPAIR], [3, 2]])
    nc.vector.tensor_reduce(
        e12_dst, DAB[:].rearrange("p s j (t c) -> p (s j t) c", c=3), AX.X, OP.add
    )

    # EPS on tgt lengths
    tsl = rap(ETRI, 0, 1, 128, 3, [[6, NB * NPAIR], [1, 3]])
    CEPS = sb.tile([128, 1], F32, tag="CEPS")
    nc.vector.memset(CEPS[:], EPS)
    nc.scalar.activation(tsl, tsl, AF.Identity, bias=CEPS[:])

    # sort3 (both st at once); planes strided by 3
    def eplane(t, e):
        return rap(t, 0, 1, 128, e, [[3, NB * NPAIR * 2]])

    SRT = sb.tile([128, NB, NPAIR, 2, 3], F32, tag="SRT")
    TL1 = sb.tile([128, NB, NPAIR, 2], F32, tag="TL1")
    TH1 = sb.tile([128, NB, NPAIR, 2], F32, tag="TH1")
    e0, e1, e2 = eplane(ETRI, 0), eplane(ETRI, 1), eplane(ETRI, 2)
    s0, s1, s2 = eplane(SRT, 0), eplane(SRT, 1), eplane(SRT, 2)
    nc.vector.tensor_tensor(TL1[:], e0, e1, OP.min)
    nc.vector.tensor_tensor(TH1[:], e0, e1, OP.max)
    nc.vector.tensor_tensor(s0, TL1[:], e2, OP.min)
    nc.vector.tensor_tensor(TL1[:], TL1[:], e2, OP.max)
    nc.vector.tensor_tensor(s1, TH1[:], TL1[:], OP.min)
    nc.vector.tensor_tensor(s2, TH1[:], TL1[:], OP.max)

    # num/den
    S_s = rap(SRT, 0, 1, 128, 0, [[6, NB * NPAIR], [1, 3]])
    S_t = rap(SRT, 0, 1, 128, 3, [[6, NB * NPAIR], [1, 3]])
    DN = sb.tile([128, NB, NPAIR, 3], F32, tag="DN")
    SM = sb.tile([128, NB, NPAIR, 3], F32, tag="SM")
    nc.vector.tensor_tensor(DN[:], S_s, S_t, OP.subtract)
    nc.vector.tensor_tensor(SM[:], S_s, S_t, OP.add)
    nc.scalar.square(DN[:], DN[:])
    nc.scalar.square(SM[:], SM[:])
    NUM = sb.tile([128, NB, NPAIR], F32, tag="NUM")
    DEN = sb.tile([128, NB, NPAIR], F32, tag="DEN")
    nc.vector.tensor_reduce(NUM[:], DN[:].rearrange("p s j c -> p (s j) c"), AX.X, OP.add)
    nc.vector.tensor_reduce(DEN[:], SM[:].rearrange("p s j c -> p (s j) c"), AX.X, OP.add)
    NEG = NUM
    nc.vector.reciprocal(DEN[:], DEN[:])
    nc.vector.scalar_tensor_tensor(NEG[:], NUM[:], -1.0, DEN[:], OP.mult, OP.mult)

    # top-10 smallest of 45 per (p, s)
    LV1 = sb.tile([128, NB, 8], F32, tag="LV1")
    LV2 = sb.tile([128, NB, 8], F32, tag="LV2")
    for s in range(NB):
        nc.vector.max(LV1[:, s, :], NEG[:, s, :])
        nc.vector.match_replace(NEG[:, s, :], LV1[:, s, :], NEG[:, s, :], BIGNEG)
        nc.vector.max(LV2[:, s, :], NEG[:, s, :])

    V10 = sb.tile([128, NB, 10], F32, tag="V10")
    nc.scalar.copy(V10[:, :, 0:8], LV1[:])
    nc.scalar.copy(V10[:, :, 8:10], LV2[:, :, 0:2])
    LX = sb.tile([128, NB, 10], F32, tag="LX")
    nc.vector.tensor_scalar(LX[:], V10[:], -1.0, EPS, OP.mult, OP.add)
    Y = sb.tile([128, NB, 10], F32, tag="Y")
    nc.scalar.activation(Y[:], LX[:], AF.Sqrt)
    Q = sb.tile([128, NB, 10], F32, tag="Q")
    for _ in range(2):
        nc.vector.reciprocal(Q[:], Y[:])
        nc.vector.tensor_tensor(Q[:], LX[:], Q[:], OP.mult)
        nc.vector.tensor_tensor(Q[:], Y[:], Q[:], OP.add)
        nc.vector.tensor_scalar(Y[:], Q[:], 0.5, None, OP.mult)

    SUM10 = sb.tile([128, NB], F32, tag="SUM10")
    nc.vector.tensor_reduce(SUM10[:], Y[:], AX.X, OP.add)
    LOSS = sb.tile([128, NB], F32, tag="LOSS")
    nc.vector.tensor_scalar(LOSS[:], SUM10[:], 0.1, None, OP.mult)

    # batch min
    M1 = sb.tile([128, 1], F32, tag="M1")
    nc.vector.tensor_reduce(M1[:], LOSS[:], AX.X, OP.min)
    ptm = ps2.tile([1, 128], F32, tag="trpsum")
    nc.tensor.transpose(ptm[:], M1[:], ident[:])
    MR = sb.tile([1, 128], F32, tag="MR")
    nc.vector.tensor_copy(MR[:], ptm[:])
    MC = sb.tile([1, 1], F32, tag="MC")
    nc.vector.tensor_reduce(MC[:], MR[:], AX.X, OP.min)
    MB = sb.tile([128, 1], F32, tag="MB")
    nc.gpsimd.partition_broadcast(MB[:], MC[:])

    W = sb.tile([128, NB], F32, tag="W")
    nc.vector.tensor_scalar(W[:], LOSS[:], MB[:], CTH, OP.subtract, OP.is_lt)

    # out: transpose W -> WT[s, pi], then one DMA
    # n = (pi%16)*128 + 8*s + pi//16 with pi = 16m + T
    ptw = ps2.tile([16, 128], F32, tag="trpsum")
    nc.tensor.transpose(ptw[:], W[:], ident[:])
    WT = sb.tile([16, 128], F32, tag="WT")
    nc.scalar.copy(WT[:], ptw[:])
    src_ap = rap(WT, 0, 1, 16, 0, [[16, 8], [1, 16]])
    dst_ap = bass.AP(
        tensor=out_d.tensor,
        offset=out_d[b].offset,
        ap=[[8, 16], [1, 8], [128, 16]],
    )
    nc.sync.dma_start(dst_ap, src_ap)


def build_program():
    if "nc" in _CACHE:
        return _CACHE["nc"]
    nc = bacc.Bacc(
        "TRN2",
        target_bir_lowering=False,
        debug=False,
        enable_asserts=False,
        num_devices=8,
    )
    src_d = nc.dram_tensor("src", [BPC, 3, N], F32, kind="ExternalInput").ap()
    tgt_d = nc.dram_tensor("tgt", [BPC, 3, N], F32, kind="ExternalInput").ap()
    out_d = nc.dram_tensor("out", [BPC, N], F32, kind="ExternalOutput").ap()

    with tile.TileContext(nc) as tc, ExitStack() as ctx:
        sb = ctx.enter_context(tc.tile_pool(name="sb", bufs=1))
        sbk = ctx.enter_context(tc.tile_pool(name="sbk", bufs=2))
        ps1 = ctx.enter_context(tc.tile_pool(name="ps1", bufs=2, space="PSUM"))
        ps2 = ctx.enter_context(tc.tile_pool(name="ps2", bufs=4, space="PSUM"))
        pools = {"sb": sb, "sbk": sbk, "ps1": ps1, "ps2": ps2}
        ident = sb.tile([128, 128], F32, tag="ident")
        masks.make_identity(nc, ident[:])
        cst = _build_consts(tc, sb)
        sts = [
            _build_setup(ctx, tc, pools, b, src_d, tgt_d, ident, cst) for b in range(BPC)
        ]
        for b in range(BPC):
            _build_main(ctx, tc, pools, b, sts[b], cst, out_d, ident)

    nc.compile()
    _CACHE["nc"] = nc
    return nc


def kernel(**inputs):
    src = np.ascontiguousarray(np.asarray(inputs["src"], dtype=np.float32))
    tgt = np.ascontiguousarray(np.asarray(inputs["tgt"], dtype=np.float32))
    B = src.shape[0]
    ncores = 8
    bpc = B // ncores
    nc = build_program()
    in_maps = [
        {"src": src[i * bpc : (i + 1) * bpc], "tgt": tgt[i * bpc : (i + 1) * bpc]}
        for i in range(ncores)
    ]
    res = run_bass_kernel_spmd(nc, in_maps, core_ids=list(range(ncores)))
    return np.concatenate([res.results[i]["out"] for i in range(ncores)], axis=0)
